# revision 36
# baseline (speedup 1.0000x reference)
"""Grouped-experts SwiGLU FFN (MoE) on 8 Trainium2 NeuronCores.

Expert-parallel: core e owns expert e's weights and its contiguous token
slice (tokens are pre-sorted by expert).  Per core, out.T = W2 @ h where
h = silu(W1 x) * (W3 x), x [2048 dim, 2048 tok].

Stage 1 runs one level of Strassen on the stacked [W1; W3] @ x product:
A = [W1; W3] is [2816, 2048] (M-halves of 1408 = 11 clean 128-row tiles),
K = 2048 and N = 2048 both split 1024.  7 M-products instead of 8
block-products cuts stage-1 PE streaming by 12.5% (1232 vs 1408 matmuls).
The A-side combinations are folded into host weight packing; the B-side
(x) sums are 5 DVE adds per token-column-pair; M-product drains and the
C recombination ride the otherwise-idle Vector engine under the PE
shadow.  silu(C_top) * C_bot then feeds stage 2 unchanged.

Precision: fp16 operands everywhere (same PE speed as bf16, 8x lower
noise), plus e4m3 fp8 DoubleRow (2 contraction blocks per matmul) for
stage-2 h-blocks 0,1 on all tokens and blocks 2,3 on tokens 0:1024.
HW-measured rel err 1.958e-2 vs the 2e-2 budget, matching the offline
quantization simulator to 3 digits; the budget is fully spent -- one
more fp8 tile-pair would land at ~2.03e-2.  fp16 w2 is host-scaled x128
so its PSUM partials match the (4h)(32w2) fp8 partials; drains scale by
1/128.

Schedule notes (from perfetto traces): PE-gap total is <1us.  The
runtime preamble is ~7us; junk warm-up matmuls issued first get the PE
HAM clock-gate to 2.4 GHz before real operands land.  Dependency
tracking is per-TILE, so the head splits x chunk 0 into separate tiles
(the opening matmul chain waits on 768KB, not 2MB) and bulk loads
(cAhi, chunk 2) are emitted after phase A's A-tile loads so their
descriptors cannot steal bandwidth.  All PSUM drains ride the scalar
engine (the DVE was the stage-1 bottleneck at ~88% busy); M6/M7 are
never drained -- the recombination adds read them from PSUM directly.
The final token chunk runs as two half-width PSUM groups drained via
scalar+vector with DMAs on sync+gpsimd queues.

Measured: 416.6us (vs 465.2us for the direct bf16 kernel this replaced;
bf16 tensor-roofline for the direct algorithm is ~451us).
"""

import numpy as np
import ml_dtypes

import concourse.bass as bass
from concourse import bacc
import concourse.mybir as mybir
from concourse.tile import TileContext
from concourse.bass_utils import run_bass_kernel_spmd

N_TOKENS = 16384
DIM = 2048
HIDDEN = 1408
N_EXPERTS = 8
N_CORES = 8

P = 128
T = 2048                 # token capacity per core per pass
N_DN = DIM // P          # 16 k-blocks (full K)
HK = 8                   # k-blocks per Strassen half (1024/128)
N_MT = HIDDEN // P       # 11 output-row tiles per M-product
N_HT = N_MT              # 11 h blocks
N_F8F = 2                # h-blocks 0,1: fp8 DR for all tokens
N_F8H = 2                # h-blocks 2,3: fp8 DR for tokens 0:1024
N_HB = 7                 # h-blocks 4..10: fp16-only path
N_W2F = 9                # fp16 w2 blocks 2..10 (2,3 used for tokens 1024:2048)
N_DT = DIM // P          # 16 output-row tiles in stage 2
TSUB = 512
HALF = T // 2

F32 = mybir.dt.float32
F16 = mybir.dt.float16
F8E4 = mybir.dt.float8e4
SILU = mybir.ActivationFunctionType.Silu
COPY = mybir.ActivationFunctionType.Copy
DROW = mybir.MatmulPerfMode.DoubleRow
ADD = mybir.AluOpType.add
SUB = mybir.AluOpType.subtract
NPF16 = np.float16
F8 = ml_dtypes.float8_e4m3
H8_SCALE = 4.0           # ht8 = e4m3(4h)
W28_SCALE = 32.0         # fp8 w2 blocks x32 -> PSUM partial 128*out
W2F_SCALE = 128.0        # fp16 w2 x128 -> matches fp8 partial scale

# Strassen M-product index order within phase B (phase A runs m2 alone):
# m4 first (its T-sum needs only x chunk A), then m5 (raw), m1/m3/m6/m7.
M2 = 1                   # m-index (0-based) of M2 = (A21+A22) B11
PHASE_B = [3, 4, 0, 2, 5, 6]   # m4, m5, m1, m3, m6, m7
N_WU = 10                # warm-up matmuls during the runtime preamble


def _build_program() -> bass.Bass:
    nc = bacc.Bacc()
    # A combinations: ap[p, mt, m, kb, c] = A_m[mt*128+c, kb*128+p]
    ap = nc.declare_dram_parameter("ap", [P, N_MT, 7, HK, P], F16, isOutput=False)
    xtp = nc.declare_dram_parameter(
        "xtp", [P, T // TSUB, N_DN, TSUB], F16, isOutput=False)
    w2p = nc.declare_dram_parameter("w2p", [P, N_DT, N_W2F, P], F16, isOutput=False)
    w28p = nc.declare_dram_parameter(
        "w28p", [P, N_DT, N_F8F + N_F8H, P], F8E4, isOutput=False)
    outt = nc.declare_dram_parameter("outt", [DIM, T], F16, isOutput=True)

    with TileContext(nc) as tc:
        with (
            tc.tile_pool(name="wu", bufs=1) as wu_pool,
            tc.tile_pool(name="xt", bufs=1) as xt_pool,
            tc.tile_pool(name="at", bufs=8) as at_pool,
            tc.tile_pool(name="tt", bufs=1) as tt_pool,
            tc.tile_pool(name="m2s", bufs=1) as m2s_pool,
            tc.tile_pool(name="ms", bufs=2) as ms_pool,
            tc.tile_pool(name="cc", bufs=1) as cc_pool,
            tc.tile_pool(name="ht", bufs=1) as ht_pool,
            tc.tile_pool(name="w2", bufs=2) as w2_pool,
            tc.tile_pool(name="w28", bufs=2) as w28_pool,
            tc.tile_pool(name="tmp", bufs=4) as tmp_pool,
            tc.tile_pool(name="ob", bufs=3) as ob_pool,
            tc.tile_pool(name="ps", bufs=1, space="PSUM") as ps_pool,
        ):
            # ---- PE warm-up: junk matmuls issued before any DMA lands so
            # the HAM clock-gate reaches 2.4 GHz during the ~7us preamble.
            # memset on gpsimd: its sequencer is ready ~2us before vector's.
            wu = wu_pool.tile([P, TSUB], F16)
            nc.gpsimd.memset(wu[:], 0.0)
            for _ in range(N_WU):
                psw = ps_pool.tile([P, TSUB], F32, bufs=4, name="psm")
                nc.tensor.matmul(psw[:], lhsT=wu[:, 0:P], rhs=wu[:],
                                 start=True, stop=True)

            # ---- bulk input tiles: chunk pair (cA, cB) per column pass;
            # p=1 reuses p=0's buffers (WAR clears once p=0 stops reading)
            # cA is split into three tiles: dependency tracking is per-tile,
            # so the opening matmul chain only waits on the 512KB it needs.
            def chunk_tiles():
                cAlo1 = xt_pool.tile([P, HK // 2, TSUB], F16, bufs=1,
                                     tag="xta1", name="xcAlo1")
                cAlo2 = xt_pool.tile([P, HK // 2, TSUB], F16, bufs=1,
                                     tag="xta2", name="xcAlo2")
                cAhi = xt_pool.tile([P, HK, TSUB], F16, bufs=1,
                                    tag="xth", name="xcAhi")
                cB = xt_pool.tile([P, N_DN, TSUB], F16, bufs=1, tag="xtb",
                                  name="xcB")
                return cAlo1, cAlo2, cAhi, cB

            def load_a(mt, m):
                at = at_pool.tile([P, HK, P], F16, tag="at")
                nc.sync.dma_start(out=at[:], in_=ap[:, mt, m, :, :])
                return at

            # Head-critical DMA order: A(mt0,m2), x chunk0 (fine-grained so
            # the first matmul chain starts ASAP), more A(m2), x chunk2.
            # Head-critical descriptors only, finest first: the opening
            # matmul chain needs A2[mt0,kb0-1] (64KB) + x[kb0-1] (256KB);
            # deps are per-tile so those are dedicated small tiles.
            # Everything else (cAhi, chunk 2) is emitted after phase A's
            # A-tile loads so it cannot steal bandwidth from them.
            cAlo1_0, cAlo2_0, cAhi0, cB0 = chunk_tiles()
            a0a = at_pool.tile([P, HK // 2, P], F16, tag="a0a", name="a0a")
            a0b = at_pool.tile([P, HK // 2, P], F16, tag="a0b", name="a0b")
            xq1 = xt_pool.tile([P, 2, TSUB], F16, bufs=1, tag="xq1", name="xq1")
            xq2 = xt_pool.tile([P, 2, TSUB], F16, bufs=1, tag="xq2", name="xq2")
            nc.sync.dma_start(out=a0a[:], in_=ap[:, 0, M2, 0:HK // 2, :])
            nc.sync.dma_start(out=xq1[:], in_=xtp[:, 0, 0:2, :])
            nc.sync.dma_start(out=a0b[:], in_=ap[:, 0, M2, HK // 2:, :])
            nc.sync.dma_start(out=xq2[:], in_=xtp[:, 0, 2:4, :])
            a2_tiles = {}
            a2_tiles[1] = load_a(1, M2)
            nc.sync.dma_start(out=cAlo2_0[:], in_=xtp[:, 0, HK // 2:HK, :])
            a2_tiles[2] = load_a(2, M2)
            a2_tiles[3] = load_a(3, M2)
            a2_tiles[4] = load_a(4, M2)
            # kb 0-3 of chunk 0 also land in cAlo1 for the T sums, which
            # run much later (p=0 matmul chains read the xq tiles instead)
            nc.sync.dma_start(out=cAlo1_0[:], in_=xtp[:, 0, 0:HK // 2, :])

            # ---- stage-1 outputs
            ht = ht_pool.tile([P, N_HB, T], F16)                 # blocks 4..10
            ht23 = ht_pool.tile([P, N_F8H, HALF], F16, name="ht23")  # 2,3 hi-half
            ht8f = ht_pool.tile([P, N_F8F, T], F8E4, name="ht8f")    # 0,1 all
            ht8h = ht_pool.tile([P, N_F8H, HALF], F8E4, name="ht8h")  # 2,3 lo-half

            # ---- stage 1: two column-pair passes (p=0: chunks 0/2, p=1: 1/3)
            for p in range(2):
                if p == 0:
                    cAlo1, cAlo2, cAhi, cB = cAlo1_0, cAlo2_0, cAhi0, cB0
                else:
                    cAlo1, cAlo2, cAhi, cB = chunk_tiles()
                    nc.sync.dma_start(out=cAlo1[:],
                                      in_=xtp[:, 1, 0:HK // 2, :])
                    nc.sync.dma_start(out=cAlo2[:],
                                      in_=xtp[:, 1, HK // 2:HK, :])
                    nc.sync.dma_start(out=cAhi[:], in_=xtp[:, 1, HK:, :])
                m2st = m2s_pool.tile([P, N_MT, TSUB], F16, tag="m2s")
                tt = tt_pool.tile([P, 5, HK, TSUB], F16, tag="tt")

                def calo(kb):
                    if p == 0 and kb < HK // 2:
                        return (xq1[:, kb, :] if kb < 2
                                else xq2[:, kb - 2, :])
                    return (cAlo1[:, kb, :] if kb < HK // 2
                            else cAlo2[:, kb - HK // 2, :])

                # phase A: M2 = (A21+A22) @ B11 for all mt (no DVE deps)
                for mt in range(N_MT):
                    if p == 0 and mt == 0:
                        def a2w(kb):
                            return (a0a[:, kb, :] if kb < HK // 2
                                    else a0b[:, kb - HK // 2, :])
                    else:
                        if p == 0 and mt in a2_tiles:
                            at = a2_tiles[mt]
                        else:
                            at = load_a(mt, M2)

                        def a2w(kb, _at=at):
                            return _at[:, kb, :]
                    psm = ps_pool.tile([P, TSUB], F32, bufs=4, name="psm")
                    for kb in range(HK):
                        nc.tensor.matmul(psm[:], lhsT=a2w(kb),
                                         rhs=calo(kb),
                                         start=(kb == 0), stop=(kb == HK - 1))
                    # drain on the (idle) scalar engine -- keeps DVE free
                    nc.scalar.activation(m2st[:, mt, :], psm[:], COPY)

                # bulk loads for this pass, behind the phase-A A-tiles
                if p == 0:
                    nc.sync.dma_start(out=cAhi[:], in_=xtp[:, 0, HK:, :])
                    for h in range(2):
                        nc.sync.dma_start(out=cB[:, h * HK:(h + 1) * HK, :],
                                          in_=xtp[:, 2, h * HK:(h + 1) * HK, :])
                else:
                    # chunk 3 into cB (WAR: after p=0's last B22 matmul)
                    nc.sync.dma_start(out=cB[:], in_=xtp[:, 3, :, :])
                # T sums (vector FIFO is otherwise idle during phase A, so
                # these run as soon as their chunks land; phase B's first
                # product needs T4)
                # T4 = B21 - B11
                nc.vector.tensor_tensor(tt[:, 1, 0:HK // 2, :],
                                        cAhi[:, 0:HK // 2, :], cAlo1[:], SUB)
                nc.vector.tensor_tensor(tt[:, 1, HK // 2:, :],
                                        cAhi[:, HK // 2:, :], cAlo2[:], SUB)
                # T1 = B11 + B22
                nc.vector.tensor_tensor(tt[:, 0, 0:HK // 2, :], cAlo1[:],
                                        cB[:, HK:HK + HK // 2, :], ADD)
                nc.vector.tensor_tensor(tt[:, 0, HK // 2:, :], cAlo2[:],
                                        cB[:, HK + HK // 2:, :], ADD)
                # T3 = B12 - B22
                nc.vector.tensor_tensor(tt[:, 2, :, :], cB[:, 0:HK, :],
                                        cB[:, HK:, :], SUB)
                # T6 = B11 + B12
                nc.vector.tensor_tensor(tt[:, 3, 0:HK // 2, :], cAlo1[:],
                                        cB[:, 0:HK // 2, :], ADD)
                nc.vector.tensor_tensor(tt[:, 3, HK // 2:, :], cAlo2[:],
                                        cB[:, HK // 2:HK, :], ADD)
                # T7 = B21 + B22
                nc.vector.tensor_tensor(tt[:, 4, :, :], cAhi[:],
                                        cB[:, HK:, :], ADD)

                rhs_by_m = {
                    0: tt[:, 0, :, :],      # M1: T1
                    2: tt[:, 2, :, :],      # M3: T3
                    3: tt[:, 1, :, :],      # M4: T4
                    4: cB[:, HK:, :],       # M5: B22 raw
                    5: tt[:, 3, :, :],      # M6: T6
                    6: tt[:, 4, :, :],      # M7: T7
                }

                # phase B: remaining 6 products per mt + recombine + swiglu.
                # M4/M5/M1/M3 drain to SBUF via scalar; M6/M7 stay in PSUM
                # and are consumed directly by the recombination adds.
                for mt in range(N_MT):
                    mts = ms_pool.tile([P, 4, TSUB], F16, tag="ms")
                    ps_keep = {}
                    for j, m in enumerate(PHASE_B):
                        at = load_a(mt, m)
                        psm = ps_pool.tile([P, TSUB], F32, bufs=4, name="psm")
                        rhs = rhs_by_m[m]
                        for kb in range(HK):
                            nc.tensor.matmul(psm[:], lhsT=at[:, kb, :],
                                             rhs=rhs[:, kb, :],
                                             start=(kb == 0),
                                             stop=(kb == HK - 1))
                        if j < 4:
                            nc.scalar.activation(mts[:, j, :], psm[:], COPY)
                        else:
                            ps_keep[m] = psm
                    m1 = mts[:, 2, :]
                    m3 = mts[:, 3, :]
                    m4 = mts[:, 0, :]
                    m5 = mts[:, 1, :]
                    m6 = ps_keep[5][:]
                    m7 = ps_keep[6][:]
                    m2 = m2st[:, mt, :]
                    cc = cc_pool.tile([P, 6, TSUB], F32, tag="cc")
                    c11, c12, c21, c22, s0, s1 = (cc[:, i, :] for i in range(6))
                    # C11 = M1 + M4 - M5 + M7  (no in-place DVE ops)
                    nc.vector.tensor_tensor(s0, m1, m4, ADD)
                    nc.vector.tensor_tensor(s1, s0, m5, SUB)
                    nc.vector.tensor_tensor(c11, s1, m7, ADD)
                    # C21 = M2 + M4
                    nc.vector.tensor_tensor(c21, m2, m4, ADD)
                    # C12 = M3 + M5
                    nc.vector.tensor_tensor(c12, m3, m5, ADD)
                    # C22 = M1 - M2 + M3 + M6
                    nc.vector.tensor_tensor(s0, m1, m2, SUB)
                    nc.vector.tensor_tensor(s1, s0, m3, ADD)
                    nc.vector.tensor_tensor(c22, s1, m6, ADD)

                    lo = p * TSUB            # token cols within each half
                    # half 1 (tokens 0:1024): fp8 for mt<4, fp16 otherwise
                    tmp = tmp_pool.tile([P, TSUB], F32, tag="tmp")
                    nc.scalar.activation(tmp[:], c11, SILU)
                    if mt < 2:
                        bsc = tmp_pool.tile([P, TSUB], F32, tag="tmp")
                        nc.scalar.activation(bsc[:], c21, COPY, scale=H8_SCALE)
                        nc.vector.tensor_tensor(
                            ht8f[:, mt, lo:lo + TSUB], tmp[:], bsc[:],
                            mybir.AluOpType.mult)
                    elif mt < 4:
                        bsc = tmp_pool.tile([P, TSUB], F32, tag="tmp")
                        nc.scalar.activation(bsc[:], c21, COPY, scale=H8_SCALE)
                        nc.vector.tensor_tensor(
                            ht8h[:, mt - 2, lo:lo + TSUB], tmp[:], bsc[:],
                            mybir.AluOpType.mult)
                    else:
                        nc.vector.tensor_tensor(
                            ht[:, mt - 4, lo:lo + TSUB], tmp[:], c21,
                            mybir.AluOpType.mult)
                    # half 2 (tokens 1024:2048): fp8 only for mt<2
                    tmp2 = tmp_pool.tile([P, TSUB], F32, tag="tmp")
                    nc.scalar.activation(tmp2[:], c12, SILU)
                    if mt < 2:
                        bsc2 = tmp_pool.tile([P, TSUB], F32, tag="tmp")
                        nc.scalar.activation(bsc2[:], c22, COPY, scale=H8_SCALE)
                        nc.vector.tensor_tensor(
                            ht8f[:, mt, HALF + lo:HALF + lo + TSUB],
                            tmp2[:], bsc2[:], mybir.AluOpType.mult)
                    elif mt < 4:
                        nc.vector.tensor_tensor(
                            ht23[:, mt - 2, lo:lo + TSUB], tmp2[:], c22,
                            mybir.AluOpType.mult)
                    else:
                        nc.vector.tensor_tensor(
                            ht[:, mt - 4, HALF + lo:HALF + lo + TSUB],
                            tmp2[:], c22, mybir.AluOpType.mult)

            # ---- stage 2: OUT.T[d, t] = sum_h W2T[h, d] * HT[h, t]
            NTS = T // TSUB
            for idt in range(N_DT):
                w2i = w2_pool.tile([P, N_W2F, P], F16, tag="w2")
                nc.sync.dma_start(out=w2i[:], in_=w2p[:, idt, :, :])
                w28i = w28_pool.tile([P, N_F8F + N_F8H, P], F8E4, tag="w28")
                nc.sync.dma_start(out=w28i[:], in_=w28p[:, idt, :, :])
                ob = ob_pool.tile([P, T], F16)
                for its in range(NTS):
                    last = (idt == N_DT - 1 and its == NTS - 1)
                    # the final token chunk runs as two half-width PSUM
                    # groups so only 256 columns of work remain after the
                    # very last matmul; the two drains ride scalar+vector
                    # and sync+gpsimd queues in parallel
                    halves = (((0, TSUB),) if not last
                              else ((0, 384), (384, TSUB)))
                    for hv, (c0, c1) in enumerate(halves):
                        seg = slice(its * TSUB + c0, its * TSUB + c1)
                        w = c1 - c0
                        pso = ps_pool.tile([P, TSUB], F32, bufs=4, name="pso")
                        po = pso[:, 0:w]
                        nc.tensor.matmul(po, lhsT=w28i[:, 0:2, :],
                                         rhs=ht8f[:, :, seg], start=True,
                                         stop=False, perf_mode=DROW)
                        if its < 2:
                            nc.tensor.matmul(po, lhsT=w28i[:, 2:4, :],
                                             rhs=ht8h[:, :, seg], start=False,
                                             stop=False, perf_mode=DROW)
                        else:
                            hseg = slice(seg.start - HALF, seg.stop - HALF)
                            for hb in range(N_F8H):
                                nc.tensor.matmul(po, lhsT=w2i[:, hb, :],
                                                 rhs=ht23[:, hb, hseg],
                                                 start=False, stop=False)
                        for j in range(N_HB):
                            nc.tensor.matmul(po, lhsT=w2i[:, N_F8H + j, :],
                                             rhs=ht[:, j, seg], start=False,
                                             stop=(j == N_HB - 1))
                        if not last:
                            nc.scalar.activation(ob[:, seg], po, COPY,
                                                 scale=1.0 / W2F_SCALE)
                            if idt == N_DT - 1:
                                nc.sync.dma_start(
                                    out=outt[idt * P:(idt + 1) * P, seg],
                                    in_=ob[:, seg])
                        elif hv == 0:
                            nc.scalar.activation(ob[:, seg], po, COPY,
                                                 scale=1.0 / W2F_SCALE)
                            nc.sync.dma_start(
                                out=outt[idt * P:(idt + 1) * P, seg],
                                in_=ob[:, seg])
                        else:
                            nc.vector.tensor_scalar_mul(ob[:, seg], po,
                                                        1.0 / W2F_SCALE)
                            nc.gpsimd.dma_start(
                                out=outt[idt * P:(idt + 1) * P, seg],
                                in_=ob[:, seg])
                if idt < N_DT - 1:
                    # idt 13's drain rides gpsimd: warms that DMA queue so
                    # the final 128-col drain (also on gpsimd) starts fast
                    eng = nc.gpsimd if idt == N_DT - 3 else nc.sync
                    eng.dma_start(out=outt[idt * P:(idt + 1) * P, :],
                                  in_=ob[:])
    nc.compile()
    return nc


_CACHE: dict = {}


def _get_nc() -> bass.Bass:
    if "nc" not in _CACHE:
        _CACHE["nc"] = _build_program()
    return _CACHE["nc"]


def _pack_weights(w1, w2, w3):
    maps = []
    for e in range(N_EXPERTS):
        a11 = w1[e][:, :DIM // 2]
        a12 = w1[e][:, DIM // 2:]
        a21 = w3[e][:, :DIM // 2]
        a22 = w3[e][:, DIM // 2:]
        am = np.stack([a11 + a22, a21 + a22, a11, a22,
                       a11 + a12, a21 - a11, a12 - a22], 0)  # [7, 1408, 1024]
        apk = np.ascontiguousarray(
            am.reshape(7, N_MT, P, HK, P).transpose(4, 1, 0, 3, 2)
            .astype(NPF16))
        w2f = np.ascontiguousarray(
            (W2F_SCALE * w2[e][:, N_F8F * P:])
            .reshape(N_DT, P, N_W2F, P).transpose(3, 0, 2, 1).astype(NPF16))
        w28 = np.clip(w2[e][:, :(N_F8F + N_F8H) * P] * W28_SCALE, -240.0, 240.0)
        w28 = np.ascontiguousarray(
            w28.reshape(N_DT, P, N_F8F + N_F8H, P).transpose(3, 0, 2, 1)
            .astype(F8))
        maps.append({"ap": apk, "w2p": w2f, "w28p": w28})
    return maps


def kernel(x, w1, w2, w3, num_tokens_per_expert, _trace=False):
    x = np.ascontiguousarray(np.asarray(x, dtype=np.float32))
    w1 = np.ascontiguousarray(np.asarray(w1, dtype=np.float32))
    w2 = np.ascontiguousarray(np.asarray(w2, dtype=np.float32))
    w3 = np.ascontiguousarray(np.asarray(w3, dtype=np.float32))
    counts = np.asarray(num_tokens_per_expert, dtype=np.int64)

    cs = np.cumsum(counts)
    starts = np.minimum(np.concatenate([[0], cs[:-1]]), N_TOKENS)
    ends = np.minimum(cs, N_TOKENS)
    lens = np.maximum(ends - starts, 0)

    wmaps = _pack_weights(w1, w2, w3)
    out = np.zeros((N_TOKENS, DIM), np.float32)
    trace_info = []

    n_passes = max(1, int(np.max(np.ceil(lens / T))))
    for k in range(n_passes):
        in_maps = []
        for e in range(N_EXPERTS):
            s = int(starts[e]) + k * T
            xe = np.zeros((T, DIM), np.float32)
            avail = x[s:s + T]
            if avail.shape[0]:
                xe[:avail.shape[0]] = avail
            xtp = np.ascontiguousarray(
                xe.T.reshape(N_DN, P, T // TSUB, TSUB)
                .transpose(1, 2, 0, 3).astype(NPF16))
            in_maps.append({"xtp": xtp, **wmaps[e]})
        res = run_bass_kernel_spmd(
            _get_nc(), in_maps, list(range(N_CORES)), trace=_trace
        )
        if _trace:
            trace_info.append(res)
        for e in range(N_EXPERTS):
            s = int(starts[e]) + k * T
            cnt = min(int(ends[e]) - s, T)
            if cnt > 0:
                out[s:s + cnt] = res.results[e]["outt"].T[:cnt].astype(np.float32)

    if _trace:
        return out, trace_info
    return out


# revision 39
# speedup vs baseline: 1.0092x; 1.0092x over previous
"""Grouped-experts SwiGLU FFN (MoE) on 8 Trainium2 NeuronCores.

Expert-parallel: core e owns expert e's weights and its contiguous token
slice (tokens are pre-sorted by expert).  Per core, out.T = W2 @ h where
h = silu(W1 x) * (W3 x), x [2048 dim, 2048 tok].

Stage 1 runs one level of Strassen on the stacked [W1; W3] @ x product:
A = [W1; W3] is [2816, 2048] (M-halves of 1408 = 11 clean 128-row tiles),
K = 2048 and N = 2048 both split 1024.  7 M-products instead of 8
block-products cuts stage-1 PE streaming by 12.5% (1232 vs 1408 matmuls).
The A-side combinations are folded into host weight packing; the B-side
(x) sums are 5 DVE adds per token-column-pair; M-product drains and the
C recombination ride the otherwise-idle Vector engine under the PE
shadow.  silu(C_top) * C_bot then feeds stage 2 unchanged.

Precision: fp16 operands everywhere (same PE speed as bf16, 8x lower
noise), plus e4m3 fp8 DoubleRow (2 contraction blocks per matmul) for
stage-2 h-blocks 0,1 on all tokens and blocks 2,3 on tokens 0:1024.
HW-measured rel err 1.958e-2 vs the 2e-2 budget, matching the offline
quantization simulator to 3 digits; the budget is fully spent -- one
more fp8 tile-pair would land at ~2.03e-2.  fp16 w2 is host-scaled x128
so its PSUM partials match the (4h)(32w2) fp8 partials; drains scale by
1/128.

Schedule notes (from perfetto traces): PE-gap total is <1us.  The
runtime preamble is ~7us; junk warm-up matmuls issued first get the PE
HAM clock-gate to 2.4 GHz before real operands land.  Dependency
tracking is per-TILE, so the head splits x chunk 0 into separate tiles
(the opening matmul chain waits on 768KB, not 2MB) and bulk loads
(cAhi, chunk 2) are emitted after phase A's A-tile loads so their
descriptors cannot steal bandwidth.  All PSUM drains ride the scalar
engine (the DVE was the stage-1 bottleneck at ~88% busy); M6/M7 are
never drained -- the recombination adds read them from PSUM directly.
The final token chunk runs as two half-width PSUM groups drained via
scalar+vector with DMAs on sync+gpsimd queues.

Measured: 416.6us (vs 465.2us for the direct bf16 kernel this replaced;
bf16 tensor-roofline for the direct algorithm is ~451us).
"""

import numpy as np
import ml_dtypes

import concourse.bass as bass
from concourse import bacc
import concourse.mybir as mybir
from concourse.tile import TileContext
from concourse.bass_utils import run_bass_kernel_spmd

N_TOKENS = 16384
DIM = 2048
HIDDEN = 1408
N_EXPERTS = 8
N_CORES = 8

P = 128
T = 2048                 # token capacity per core per pass
N_DN = DIM // P          # 16 k-blocks (full K)
HK = 8                   # k-blocks per Strassen half (1024/128)
N_MT = HIDDEN // P       # 11 output-row tiles per M-product
N_HT = N_MT              # 11 h blocks
N_F8F = 2                # h-blocks 0,1: fp8 DR for all tokens
N_F8H = 2                # h-blocks 2,3: fp8 DR for tokens 0:1024
N_HB = 7                 # h-blocks 4..10: fp16-only path
N_W2F = 9                # fp16 w2 blocks 2..10 (2,3 used for tokens 1024:2048)
N_DT = DIM // P          # 16 output-row tiles in stage 2
TSUB = 512
HALF = T // 2

F32 = mybir.dt.float32
F16 = mybir.dt.float16
F8E4 = mybir.dt.float8e4
SILU = mybir.ActivationFunctionType.Silu
COPY = mybir.ActivationFunctionType.Copy
DROW = mybir.MatmulPerfMode.DoubleRow
ADD = mybir.AluOpType.add
SUB = mybir.AluOpType.subtract
NPF16 = np.float16
F8 = ml_dtypes.float8_e4m3
H8_SCALE = 4.0           # ht8 = e4m3(4h)
W28_SCALE = 32.0         # fp8 w2 blocks x32 -> PSUM partial 128*out
W2F_SCALE = 128.0        # fp16 w2 x128 -> matches fp8 partial scale

# Strassen M-product index order within phase B (phase A runs m2 alone):
# m4 first (its T-sum needs only x chunk A), then m5 (raw), m1/m3/m6/m7.
M2 = 1                   # m-index (0-based) of M2 = (A21+A22) B11
PHASE_B = [3, 4, 0, 2, 5, 6]   # m4, m5, m1, m3, m6, m7
N_WU = 10                # warm-up matmuls during the runtime preamble


def _build_program() -> bass.Bass:
    nc = bacc.Bacc()
    # A combinations: ap[p, mt, m, kb, c] = A_m[mt*128+c, kb*128+p]
    ap = nc.declare_dram_parameter("ap", [P, N_MT, 7, HK, P], F16, isOutput=False)
    xtp = nc.declare_dram_parameter(
        "xtp", [P, T // TSUB, N_DN, TSUB], F16, isOutput=False)
    w2p = nc.declare_dram_parameter("w2p", [P, N_DT, N_W2F, P], F16, isOutput=False)
    w28p = nc.declare_dram_parameter(
        "w28p", [P, N_DT, N_F8F + N_F8H, P], F8E4, isOutput=False)
    outt = nc.declare_dram_parameter("outt", [DIM, T], F16, isOutput=True)

    with TileContext(nc) as tc:
        with (
            tc.tile_pool(name="wu", bufs=1) as wu_pool,
            tc.tile_pool(name="xt", bufs=1) as xt_pool,
            tc.tile_pool(name="at", bufs=8) as at_pool,
            tc.tile_pool(name="tt", bufs=1) as tt_pool,
            tc.tile_pool(name="m2s", bufs=1) as m2s_pool,
            tc.tile_pool(name="ms", bufs=2) as ms_pool,
            tc.tile_pool(name="cc", bufs=1) as cc_pool,
            tc.tile_pool(name="ht", bufs=1) as ht_pool,
            tc.tile_pool(name="w2", bufs=2) as w2_pool,
            tc.tile_pool(name="w28", bufs=2) as w28_pool,
            tc.tile_pool(name="tmp", bufs=4) as tmp_pool,
            tc.tile_pool(name="ob", bufs=3) as ob_pool,
            tc.tile_pool(name="ps", bufs=1, space="PSUM") as ps_pool,
        ):
            # ---- PE warm-up: junk matmuls issued before any DMA lands so
            # the HAM clock-gate reaches 2.4 GHz during the ~7us preamble.
            # memset on gpsimd: its sequencer is ready ~2us before vector's.
            wu = wu_pool.tile([P, TSUB], F16)
            nc.gpsimd.memset(wu[:], 0.0)
            for _ in range(N_WU):
                psw = ps_pool.tile([P, TSUB], F32, bufs=4, name="psm")
                nc.tensor.matmul(psw[:], lhsT=wu[:, 0:P], rhs=wu[:],
                                 start=True, stop=True)

            # ---- bulk input tiles: chunk pair (cA, cB) per column pass;
            # p=1 reuses p=0's buffers (WAR clears once p=0 stops reading)
            # cA is split into three tiles: dependency tracking is per-tile,
            # so the opening matmul chain only waits on the 512KB it needs.
            def chunk_tiles():
                cAlo1 = xt_pool.tile([P, HK // 2, TSUB], F16, bufs=1,
                                     tag="xta1", name="xcAlo1")
                cAlo2 = xt_pool.tile([P, HK // 2, TSUB], F16, bufs=1,
                                     tag="xta2", name="xcAlo2")
                cAhi = xt_pool.tile([P, HK, TSUB], F16, bufs=1,
                                    tag="xth", name="xcAhi")
                cB = xt_pool.tile([P, N_DN, TSUB], F16, bufs=1, tag="xtb",
                                  name="xcB")
                return cAlo1, cAlo2, cAhi, cB

            def load_a(mt, m):
                at = at_pool.tile([P, HK, P], F16, tag="at")
                nc.sync.dma_start(out=at[:], in_=ap[:, mt, m, :, :])
                return at

            # Head-critical DMA order: A(mt0,m2), x chunk0 (fine-grained so
            # the first matmul chain starts ASAP), more A(m2), x chunk2.
            # Head-critical descriptors only: A2(mt0) + B11 of chunk 0 feed
            # the opening matmul chains.  Everything else (cAhi, chunk 2)
            # is emitted after phase A's A-tile loads so it cannot steal
            # bandwidth from them.
            cAlo1_0, cAlo2_0, cAhi0, cB0 = chunk_tiles()
            a2_tiles = {}
            a2_tiles[0] = load_a(0, M2)
            nc.sync.dma_start(out=cAlo1_0[:], in_=xtp[:, 0, 0:HK // 2, :])
            a2_tiles[1] = load_a(1, M2)
            nc.sync.dma_start(out=cAlo2_0[:], in_=xtp[:, 0, HK // 2:HK, :])
            a2_tiles[2] = load_a(2, M2)
            a2_tiles[3] = load_a(3, M2)
            a2_tiles[4] = load_a(4, M2)

            # ---- stage-1 outputs
            ht = ht_pool.tile([P, N_HB, T], F16)                 # blocks 4..10
            ht23 = ht_pool.tile([P, N_F8H, HALF], F16, name="ht23")  # 2,3 hi-half
            ht8f = ht_pool.tile([P, N_F8F, T], F8E4, name="ht8f")    # 0,1 all
            ht8h = ht_pool.tile([P, N_F8H, HALF], F8E4, name="ht8h")  # 2,3 lo-half

            # ---- stage 1: two column-pair passes (p=0: chunks 0/2, p=1: 1/3)
            for p in range(2):
                if p == 0:
                    cAlo1, cAlo2, cAhi, cB = cAlo1_0, cAlo2_0, cAhi0, cB0
                else:
                    cAlo1, cAlo2, cAhi, cB = chunk_tiles()
                    nc.sync.dma_start(out=cAlo1[:],
                                      in_=xtp[:, 1, 0:HK // 2, :])
                    nc.sync.dma_start(out=cAlo2[:],
                                      in_=xtp[:, 1, HK // 2:HK, :])
                    nc.sync.dma_start(out=cAhi[:], in_=xtp[:, 1, HK:, :])
                m2st = m2s_pool.tile([P, N_MT, TSUB], F16, tag="m2s")
                tt = tt_pool.tile([P, 5, HK, TSUB], F16, tag="tt")

                def calo(kb):
                    return (cAlo1[:, kb, :] if kb < HK // 2
                            else cAlo2[:, kb - HK // 2, :])

                # phase A: M2 = (A21+A22) @ B11 for all mt (no DVE deps)
                for mt in range(N_MT):
                    if p == 0 and mt in a2_tiles:
                        at = a2_tiles[mt]
                    else:
                        at = load_a(mt, M2)
                    psm = ps_pool.tile([P, TSUB], F32, bufs=4, name="psm")
                    for kb in range(HK):
                        nc.tensor.matmul(psm[:], lhsT=at[:, kb, :],
                                         rhs=calo(kb),
                                         start=(kb == 0), stop=(kb == HK - 1))
                    # drain on the (idle) scalar engine -- keeps DVE free
                    nc.scalar.activation(m2st[:, mt, :], psm[:], COPY)

                # bulk loads for this pass, behind the phase-A A-tiles
                if p == 0:
                    nc.sync.dma_start(out=cAhi[:], in_=xtp[:, 0, HK:, :])
                    for h in range(2):
                        nc.sync.dma_start(out=cB[:, h * HK:(h + 1) * HK, :],
                                          in_=xtp[:, 2, h * HK:(h + 1) * HK, :])
                else:
                    # chunk 3 into cB (WAR: after p=0's last B22 matmul)
                    nc.sync.dma_start(out=cB[:], in_=xtp[:, 3, :, :])
                # T sums (vector FIFO is otherwise idle during phase A, so
                # these run as soon as their chunks land; phase B's first
                # product needs T4)
                # T4 = B21 - B11
                nc.vector.tensor_tensor(tt[:, 1, 0:HK // 2, :],
                                        cAhi[:, 0:HK // 2, :], cAlo1[:], SUB)
                nc.vector.tensor_tensor(tt[:, 1, HK // 2:, :],
                                        cAhi[:, HK // 2:, :], cAlo2[:], SUB)
                # T1 = B11 + B22
                nc.vector.tensor_tensor(tt[:, 0, 0:HK // 2, :], cAlo1[:],
                                        cB[:, HK:HK + HK // 2, :], ADD)
                nc.vector.tensor_tensor(tt[:, 0, HK // 2:, :], cAlo2[:],
                                        cB[:, HK + HK // 2:, :], ADD)
                # T3 = B12 - B22
                nc.vector.tensor_tensor(tt[:, 2, :, :], cB[:, 0:HK, :],
                                        cB[:, HK:, :], SUB)
                # T6 = B11 + B12
                nc.vector.tensor_tensor(tt[:, 3, 0:HK // 2, :], cAlo1[:],
                                        cB[:, 0:HK // 2, :], ADD)
                nc.vector.tensor_tensor(tt[:, 3, HK // 2:, :], cAlo2[:],
                                        cB[:, HK // 2:HK, :], ADD)
                # T7 = B21 + B22
                nc.vector.tensor_tensor(tt[:, 4, :, :], cAhi[:],
                                        cB[:, HK:, :], ADD)

                rhs_by_m = {
                    0: tt[:, 0, :, :],      # M1: T1
                    2: tt[:, 2, :, :],      # M3: T3
                    3: tt[:, 1, :, :],      # M4: T4
                    4: cB[:, HK:, :],       # M5: B22 raw
                    5: tt[:, 3, :, :],      # M6: T6
                    6: tt[:, 4, :, :],      # M7: T7
                }

                # phase B: remaining 6 products per mt + recombine + swiglu.
                # M4/M5/M1/M3 drain to SBUF via scalar; M6/M7 stay in PSUM
                # and are consumed directly by the recombination adds.
                for mt in range(N_MT):
                    mts = ms_pool.tile([P, 4, TSUB], F16, tag="ms")
                    ps_keep = {}
                    for j, m in enumerate(PHASE_B):
                        at = load_a(mt, m)
                        psm = ps_pool.tile([P, TSUB], F32, bufs=4, name="psm")
                        rhs = rhs_by_m[m]
                        for kb in range(HK):
                            nc.tensor.matmul(psm[:], lhsT=at[:, kb, :],
                                             rhs=rhs[:, kb, :],
                                             start=(kb == 0),
                                             stop=(kb == HK - 1))
                        if j < 4:
                            nc.scalar.activation(mts[:, j, :], psm[:], COPY)
                        else:
                            ps_keep[m] = psm
                    m1 = mts[:, 2, :]
                    m3 = mts[:, 3, :]
                    m4 = mts[:, 0, :]
                    m5 = mts[:, 1, :]
                    m6 = ps_keep[5][:]
                    m7 = ps_keep[6][:]
                    m2 = m2st[:, mt, :]
                    cc = cc_pool.tile([P, 6, TSUB], F32, tag="cc")
                    c11, c12, c21, c22, s0, s1 = (cc[:, i, :] for i in range(6))
                    # C11 = M1 + M4 - M5 + M7  (no in-place DVE ops)
                    nc.vector.tensor_tensor(s0, m1, m4, ADD)
                    nc.vector.tensor_tensor(s1, s0, m5, SUB)
                    nc.vector.tensor_tensor(c11, s1, m7, ADD)
                    # C21 = M2 + M4
                    nc.vector.tensor_tensor(c21, m2, m4, ADD)
                    # C12 = M3 + M5
                    nc.vector.tensor_tensor(c12, m3, m5, ADD)
                    # C22 = M1 - M2 + M3 + M6
                    nc.vector.tensor_tensor(s0, m1, m2, SUB)
                    nc.vector.tensor_tensor(s1, s0, m3, ADD)
                    nc.vector.tensor_tensor(c22, s1, m6, ADD)

                    lo = p * TSUB            # token cols within each half
                    # half 1 (tokens 0:1024): fp8 for mt<4, fp16 otherwise
                    tmp = tmp_pool.tile([P, TSUB], F32, tag="tmp")
                    nc.scalar.activation(tmp[:], c11, SILU)
                    if mt < 2:
                        bsc = tmp_pool.tile([P, TSUB], F32, tag="tmp")
                        nc.scalar.activation(bsc[:], c21, COPY, scale=H8_SCALE)
                        nc.vector.tensor_tensor(
                            ht8f[:, mt, lo:lo + TSUB], tmp[:], bsc[:],
                            mybir.AluOpType.mult)
                    elif mt < 4:
                        bsc = tmp_pool.tile([P, TSUB], F32, tag="tmp")
                        nc.scalar.activation(bsc[:], c21, COPY, scale=H8_SCALE)
                        nc.vector.tensor_tensor(
                            ht8h[:, mt - 2, lo:lo + TSUB], tmp[:], bsc[:],
                            mybir.AluOpType.mult)
                    else:
                        nc.vector.tensor_tensor(
                            ht[:, mt - 4, lo:lo + TSUB], tmp[:], c21,
                            mybir.AluOpType.mult)
                    # half 2 (tokens 1024:2048): fp8 only for mt<2
                    tmp2 = tmp_pool.tile([P, TSUB], F32, tag="tmp")
                    nc.scalar.activation(tmp2[:], c12, SILU)
                    if mt < 2:
                        bsc2 = tmp_pool.tile([P, TSUB], F32, tag="tmp")
                        nc.scalar.activation(bsc2[:], c22, COPY, scale=H8_SCALE)
                        nc.vector.tensor_tensor(
                            ht8f[:, mt, HALF + lo:HALF + lo + TSUB],
                            tmp2[:], bsc2[:], mybir.AluOpType.mult)
                    elif mt < 4:
                        nc.vector.tensor_tensor(
                            ht23[:, mt - 2, lo:lo + TSUB], tmp2[:], c22,
                            mybir.AluOpType.mult)
                    else:
                        nc.vector.tensor_tensor(
                            ht[:, mt - 4, HALF + lo:HALF + lo + TSUB],
                            tmp2[:], c22, mybir.AluOpType.mult)

            # ---- stage 2: OUT.T[d, t] = sum_h W2T[h, d] * HT[h, t]
            NTS = T // TSUB
            for idt in range(N_DT):
                w2i = w2_pool.tile([P, N_W2F, P], F16, tag="w2")
                nc.sync.dma_start(out=w2i[:], in_=w2p[:, idt, :, :])
                w28i = w28_pool.tile([P, N_F8F + N_F8H, P], F8E4, tag="w28")
                nc.sync.dma_start(out=w28i[:], in_=w28p[:, idt, :, :])
                ob = ob_pool.tile([P, T], F16)
                for its in range(NTS):
                    last = (idt == N_DT - 1 and its == NTS - 1)
                    # the final token chunk runs as two half-width PSUM
                    # groups so only 256 columns of work remain after the
                    # very last matmul; the two drains ride scalar+vector
                    # and sync+gpsimd queues in parallel
                    halves = (((0, TSUB),) if not last
                              else ((0, 384), (384, TSUB)))
                    for hv, (c0, c1) in enumerate(halves):
                        seg = slice(its * TSUB + c0, its * TSUB + c1)
                        w = c1 - c0
                        pso = ps_pool.tile([P, TSUB], F32, bufs=4, name="pso")
                        po = pso[:, 0:w]
                        nc.tensor.matmul(po, lhsT=w28i[:, 0:2, :],
                                         rhs=ht8f[:, :, seg], start=True,
                                         stop=False, perf_mode=DROW)
                        if its < 2:
                            nc.tensor.matmul(po, lhsT=w28i[:, 2:4, :],
                                             rhs=ht8h[:, :, seg], start=False,
                                             stop=False, perf_mode=DROW)
                        else:
                            hseg = slice(seg.start - HALF, seg.stop - HALF)
                            for hb in range(N_F8H):
                                nc.tensor.matmul(po, lhsT=w2i[:, hb, :],
                                                 rhs=ht23[:, hb, hseg],
                                                 start=False, stop=False)
                        for j in range(N_HB):
                            nc.tensor.matmul(po, lhsT=w2i[:, N_F8H + j, :],
                                             rhs=ht[:, j, seg], start=False,
                                             stop=(j == N_HB - 1))
                        if not last:
                            nc.scalar.activation(ob[:, seg], po, COPY,
                                                 scale=1.0 / W2F_SCALE)
                            if idt == N_DT - 1:
                                nc.sync.dma_start(
                                    out=outt[idt * P:(idt + 1) * P, seg],
                                    in_=ob[:, seg])
                        elif hv == 0:
                            nc.scalar.activation(ob[:, seg], po, COPY,
                                                 scale=1.0 / W2F_SCALE)
                            nc.sync.dma_start(
                                out=outt[idt * P:(idt + 1) * P, seg],
                                in_=ob[:, seg])
                        else:
                            nc.vector.tensor_scalar_mul(ob[:, seg], po,
                                                        1.0 / W2F_SCALE)
                            nc.gpsimd.dma_start(
                                out=outt[idt * P:(idt + 1) * P, seg],
                                in_=ob[:, seg])
                if idt < N_DT - 1:
                    # idt 13's drain rides gpsimd: warms that DMA queue so
                    # the final 128-col drain (also on gpsimd) starts fast
                    eng = nc.gpsimd if idt == N_DT - 3 else nc.sync
                    eng.dma_start(out=outt[idt * P:(idt + 1) * P, :],
                                  in_=ob[:])
    nc.compile()
    return nc


_CACHE: dict = {}


def _get_nc() -> bass.Bass:
    if "nc" not in _CACHE:
        _CACHE["nc"] = _build_program()
    return _CACHE["nc"]


def _pack_weights(w1, w2, w3):
    maps = []
    for e in range(N_EXPERTS):
        a11 = w1[e][:, :DIM // 2]
        a12 = w1[e][:, DIM // 2:]
        a21 = w3[e][:, :DIM // 2]
        a22 = w3[e][:, DIM // 2:]
        am = np.stack([a11 + a22, a21 + a22, a11, a22,
                       a11 + a12, a21 - a11, a12 - a22], 0)  # [7, 1408, 1024]
        apk = np.ascontiguousarray(
            am.reshape(7, N_MT, P, HK, P).transpose(4, 1, 0, 3, 2)
            .astype(NPF16))
        w2f = np.ascontiguousarray(
            (W2F_SCALE * w2[e][:, N_F8F * P:])
            .reshape(N_DT, P, N_W2F, P).transpose(3, 0, 2, 1).astype(NPF16))
        w28 = np.clip(w2[e][:, :(N_F8F + N_F8H) * P] * W28_SCALE, -240.0, 240.0)
        w28 = np.ascontiguousarray(
            w28.reshape(N_DT, P, N_F8F + N_F8H, P).transpose(3, 0, 2, 1)
            .astype(F8))
        maps.append({"ap": apk, "w2p": w2f, "w28p": w28})
    return maps


def kernel(x, w1, w2, w3, num_tokens_per_expert, _trace=False):
    x = np.ascontiguousarray(np.asarray(x, dtype=np.float32))
    w1 = np.ascontiguousarray(np.asarray(w1, dtype=np.float32))
    w2 = np.ascontiguousarray(np.asarray(w2, dtype=np.float32))
    w3 = np.ascontiguousarray(np.asarray(w3, dtype=np.float32))
    counts = np.asarray(num_tokens_per_expert, dtype=np.int64)

    cs = np.cumsum(counts)
    starts = np.minimum(np.concatenate([[0], cs[:-1]]), N_TOKENS)
    ends = np.minimum(cs, N_TOKENS)
    lens = np.maximum(ends - starts, 0)

    wmaps = _pack_weights(w1, w2, w3)
    out = np.zeros((N_TOKENS, DIM), np.float32)
    trace_info = []

    n_passes = max(1, int(np.max(np.ceil(lens / T))))
    for k in range(n_passes):
        in_maps = []
        for e in range(N_EXPERTS):
            s = int(starts[e]) + k * T
            xe = np.zeros((T, DIM), np.float32)
            avail = x[s:s + T]
            if avail.shape[0]:
                xe[:avail.shape[0]] = avail
            xtp = np.ascontiguousarray(
                xe.T.reshape(N_DN, P, T // TSUB, TSUB)
                .transpose(1, 2, 0, 3).astype(NPF16))
            in_maps.append({"xtp": xtp, **wmaps[e]})
        res = run_bass_kernel_spmd(
            _get_nc(), in_maps, list(range(N_CORES)), trace=_trace
        )
        if _trace:
            trace_info.append(res)
        for e in range(N_EXPERTS):
            s = int(starts[e]) + k * T
            cnt = min(int(ends[e]) - s, T)
            if cnt > 0:
                out[s:s + cnt] = res.results[e]["outt"].T[:cnt].astype(np.float32)

    if _trace:
        return out, trace_info
    return out
